# revision 12
# baseline (speedup 1.0000x reference)
"""Multi-head causal self-attention block on 8 Trainium2 NeuronCores.

Reference computation (fp32):
    qkv = x @ W1.T + b1          x:(2,2048,768)  W1:(2304,768)
    q,k,v split -> 12 heads of 64
    scores = causal(q @ k.T / 8), softmax, o = attn @ v
    out = o @ W2.T + b2

Sharding: core = batch b (2) x head-group g (4, 3 heads each).
Each core computes QKV for its heads (TP columns of W1), attention, and a
partial out-projection over its 192 channels (TP rows of W2).  Host sums the
4 partials per batch (the TP all-reduce) and adds b2' = b2 + W2 @ b1_v
(the v-bias is linear through attention since softmax weights sum to 1).

v2 design (bf16 everywhere on the PE):
  - all matmul operands bf16 (PSUM accumulates fp32): sustains ~260ns per
    512-col matmul and, unlike float32r, has no 4x penalty below N=256,
    which unlocks narrow diagonal tiles.
  - contraction is exactly 768 = 6 c-tiles (no bias row): q/k bias is added
    per-partition during the PSUM->SBUF copy (tensor_scalar_add), the
    softmax-denominator ones column is memset once.
  - q|k kept packed per head in one [128, T] tile -> single copy per
    (head, m-block).
  - diagonal m-blocks narrowed: for diag key-tile p only query columns
    >= 128p are computed (QK, exp, PV), and the causal mask is one shared
    [128,128] lower-triangular additive matmul on the mixed block.
  - softmax denominator reciprocal via reciprocal_approx_fast (~0.8us vs
    4us for vector.reciprocal, which serialized the out-projection).
  - out-projection for block i is emitted in iteration i+1 (before that
    block's attention) so it never waits on the normalize chain.
"""

import os

import numpy as np
import ml_dtypes

import concourse.bass as bass
import concourse.tile as tile
from concourse import bacc
from concourse import mybir
from concourse import bass_utils

B = 2
T = 2048
C = 768
NH = 12
D = 64
NCORES = 8
GROUPS = 4               # head groups (tensor parallel)
NH_CORE = NH // GROUPS   # 3 heads per core
CC = NH_CORE * D         # 192 channels per core
MB = 512                 # query m-block width (PSUM bank)
NMB = T // MB            # 4 m-blocks
NTK = T // 128           # 16 key tiles
VW = D + 1               # v with ones column
NCT = C // 128           # 6 c-tiles
F32 = mybir.dt.float32
BF16 = mybir.dt.bfloat16
MASK_VAL = -1.0e9

LAST_RESULTS = None      # BassKernelResults of the last run (for test.py)


def _flag(name, default):
    return int(os.environ.get(name, default))


def _build_masks() -> np.ndarray:
    """[128, 256] bf16: lower-tri additive mask (keep iff key<=query) | identity."""
    out = np.zeros((128, 256), np.float32)
    r = np.arange(128)[:, None]
    c = np.arange(128)[None, :]
    out[:, 0:128] = np.where(r <= c, 0.0, MASK_VAL)
    out[:, 128:256] = np.eye(128, dtype=np.float32)
    return out.astype(ml_dtypes.bfloat16)


def _build_program() -> bass.Bass:
    pt_bufs = _flag("K_PT_BUFS", 3)
    qk_bufs = _flag("K_QK_BUFS", 2)
    pv_bufs = _flag("K_PV_BUFS", 2)
    proj_bufs = _flag("K_PROJ_BUFS", 2)

    nc = bacc.Bacc(
        "TRN2", target_bir_lowering=False, debug=False, num_devices=NCORES
    )

    debug = _flag("K_DEBUG", 0)
    x_d = nc.dram_tensor("x", (C, T), BF16, kind="ExternalInput").ap()
    w1qk_d = nc.dram_tensor("w1qk", (C, 2 * CC), BF16, kind="ExternalInput").ap()
    w1v_d = nc.dram_tensor("w1v", (C, CC), BF16, kind="ExternalInput").ap()
    w2a_d = nc.dram_tensor("w2a", (128, C), BF16, kind="ExternalInput").ap()
    w2b_d = nc.dram_tensor("w2b", (CC - 128, C), BF16, kind="ExternalInput").ap()
    mi_d = nc.dram_tensor("mi", (128, 256), BF16, kind="ExternalInput").ap()
    bqk_d = nc.dram_tensor("bqk", (128, NH_CORE), F32, kind="ExternalInput").ap()
    out_d = nc.dram_tensor("outT", (C, T), BF16, kind="ExternalOutput").ap()
    if debug:
        v_dbg = nc.dram_tensor(
            "v_dbg", (128, NTK * NH_CORE * VW), BF16, kind="ExternalOutput"
        ).ap()
        q_dbg = nc.dram_tensor("q_dbg", (D, T), BF16, kind="ExternalOutput").ap()
        k_dbg = nc.dram_tensor("k_dbg", (D, T), BF16, kind="ExternalOutput").ap()
        oa_dbg = nc.dram_tensor("oa_dbg", (128, T), BF16, kind="ExternalOutput").ap()

    with tile.TileContext(nc) as tc:
        with (
            nc.allow_low_precision(reason="bf16 matmuls within 2e-2 tolerance"),
            tc.tile_pool(name="persist", bufs=1) as persist,
            tc.tile_pool(name="pt_pool", bufs=pt_bufs) as pt_pool,
            tc.tile_pool(name="small", bufs=2) as small,
            tc.tile_pool(name="ostage", bufs=2) as ostage,
            tc.tile_pool(name="proj_ps", bufs=proj_bufs, space="PSUM") as proj_ps,
            tc.tile_pool(name="qk_ps", bufs=qk_bufs, space="PSUM") as qk_ps,
            tc.tile_pool(name="pv_ps", bufs=pv_bufs, space="PSUM") as pv_ps,
        ):
            # ---- input DMAs on two HW queues (sync + scalar) ----
            # sync: the critical path for the first qk-proj chain, interleaved
            # per c-tile so matmul ci can start as soon as its pair lands.
            xfull = []
            w1qk = []
            w1v = []
            for ci in range(NCT):
                xti = persist.tile([128, T], BF16, tag=f"x_{ci}")
                nc.sync.dma_start(
                    xti[:, 0:MB], x_d[128 * ci : 128 * (ci + 1), 0:MB]
                )
                wt = persist.tile([128, 2 * CC], BF16, tag=f"w1qk_{ci}")
                nc.sync.dma_start(wt, w1qk_d[128 * ci : 128 * (ci + 1), :])
                xfull.append(xti)
                w1qk.append(wt)
            mi = persist.tile([128, 256], BF16, tag="mi")
            nc.sync.dma_start(mi, mi_d)
            w2a = persist.tile([128, C], BF16, tag="w2a")
            nc.sync.dma_start(w2a, w2a_d)
            w2b = persist.tile([CC - 128, C], BF16, tag="w2b")
            nc.sync.dma_start(w2b, w2b_d)
            # scalar queue: bias + v weights (needed a few us in), then the
            # remaining three x quarters as one wide DMA per c-tile.
            bqk = persist.tile([128, NH_CORE], F32, tag="bqk")
            nc.scalar.dma_start(bqk, bqk_d)
            for ci in range(NCT):
                vt = persist.tile([128, CC], BF16, tag=f"w1v_{ci}")
                nc.scalar.dma_start(vt, w1v_d[128 * ci : 128 * (ci + 1), :])
                w1v.append(vt)
            for ci in range(NCT):
                nc.scalar.dma_start(
                    xfull[ci][:, MB:T], x_d[128 * ci : 128 * (ci + 1), MB:T]
                )
            xt = [
                [xfull[ci][:, MB * q : MB * (q + 1)] for ci in range(NCT)]
                for q in range(NMB)
            ]

            tri_mask = mi[:, 0:128]
            ident = mi[:, 128:256]

            # v: key-major, interleaved [tile(16), head(3), 64 v + 1 one]
            v_sb = persist.tile([128, NTK * NH_CORE * VW], BF16, tag="v_sb")
            ones_ap = v_sb.rearrange(
                "p (t h u) -> p t h u", t=NTK, h=NH_CORE
            )[:, :, :, D : D + 1]
            nc.gpsimd.memset(ones_ap, 1.0)

            # separate q/k per head (matmul needs equal base partitions);
            # q rows pre-scaled by 1/sqrt(D) host-side
            qT = [
                persist.tile([D, T], BF16, tag=f"qT{hh}", name=f"qT{hh}")
                for hh in range(NH_CORE)
            ]
            kT = [
                persist.tile([D, T], BF16, tag=f"kT{hh}", name=f"kT{hh}")
                for hh in range(NH_CORE)
            ]
            oT_a = persist.tile([128, T], BF16, tag="oT_a")  # heads 0,1
            oT_b = persist.tile([D, T], BF16, tag="oT_b")    # head 2

            def outproj(i):
                for fc in range(C // 128):
                    ps = proj_ps.tile([128, MB], F32, tag="ps")
                    nc.tensor.matmul(
                        ps,
                        lhsT=w2a[:, 128 * fc : 128 * (fc + 1)],
                        rhs=oT_a[:, MB * i : MB * (i + 1)],
                        start=True,
                        stop=False,
                    )
                    nc.tensor.matmul(
                        ps,
                        lhsT=w2b[:, 128 * fc : 128 * (fc + 1)],
                        rhs=oT_b[:, MB * i : MB * (i + 1)],
                        start=False,
                        stop=True,
                    )
                    osb = ostage.tile([128, MB], BF16, tag="osb")
                    nc.vector.tensor_copy(osb, ps)
                    nc.sync.dma_start(
                        out_d[128 * fc : 128 * (fc + 1), MB * i : MB * (i + 1)],
                        osb,
                    )

            for i in range(NMB):
                # ---- qk projection for t-quarter i ----
                for hh in range(NH_CORE):
                    ps = proj_ps.tile([128, MB], F32, tag="ps")
                    for ci in range(NCT):
                        nc.tensor.matmul(
                            ps,
                            lhsT=w1qk[ci][:, 128 * hh : 128 * (hh + 1)],
                            rhs=xt[i][ci],
                            start=(ci == 0),
                            stop=(ci == NCT - 1),
                        )
                    # q bias folded into the copy; k bias is softmax-invariant
                    # (adds a per-query constant to every logit) so dropped.
                    nc.vector.tensor_scalar_add(
                        qT[hh][:, MB * i : MB * (i + 1)],
                        ps[0:D, :],
                        bqk[0:D, hh : hh + 1],
                    )
                    nc.vector.tensor_copy(
                        kT[hh][:, MB * i : MB * (i + 1)], ps[D:128, :]
                    )
                # ---- v projection for t-chunks 4i..4i+3 ----
                for tch in range(4):
                    ps = proj_ps.tile([128, CC], F32, tag="ps")
                    for ci in range(NCT):
                        nc.tensor.matmul(
                            ps,
                            lhsT=xt[i][ci][:, 128 * tch : 128 * (tch + 1)],
                            rhs=w1v[ci],
                            start=(ci == 0),
                            stop=(ci == NCT - 1),
                        )
                    chunk = v_sb[
                        :,
                        NH_CORE * VW * (4 * i + tch) :
                        NH_CORE * VW * (4 * i + tch + 1),
                    ].rearrange("p (h u) -> p h u", h=NH_CORE)
                    nc.vector.tensor_copy(
                        chunk[:, :, 0:D],
                        ps.rearrange("p (h u) -> p h u", h=NH_CORE),
                    )

                # ---- out-projection for the previous m-block ----
                if i > 0:
                    outproj(i - 1)

                # ---- attention for m-block i ----
                def vj_ap(j, hh):
                    base = NH_CORE * VW * j + VW * hh
                    return v_sb[:, base : base + VW]

                for hh in range(NH_CORE):
                    pvps = pv_ps.tile([VW, MB], F32, tag="pv")
                    qrow = qT[hh]
                    krow = kT[hh]
                    # off-diagonal key tiles, two per PSUM tile
                    for j0 in range(0, 4 * i, 2):
                        qkps = qk_ps.tile([128, 2 * MB], F32, tag="qk")
                        for u in range(2):
                            j = j0 + u
                            nc.tensor.matmul(
                                qkps[:, MB * u : MB * (u + 1)],
                                lhsT=krow[:, 128 * j : 128 * (j + 1)],
                                rhs=qrow[:, MB * i : MB * (i + 1)],
                                start=True,
                                stop=True,
                            )
                        pt = pt_pool.tile([128, 2 * MB], BF16, tag="pt")
                        nc.scalar.activation(
                            pt, qkps, mybir.ActivationFunctionType.Exp
                        )
                        for u in range(2):
                            nc.tensor.matmul(
                                pvps,
                                lhsT=vj_ap(j0 + u, hh),
                                rhs=pt[:, MB * u : MB * (u + 1)],
                                start=(j0 + u == 0),
                                stop=False,
                                skip_group_check=True,
                            )
                    # diagonal key tiles p=0..3, narrowed to cols >= 128p,
                    # packed (p0,p1) -> tile A, (p2,p3) -> tile B
                    for pg in range(2):
                        widths = [MB - 128 * (2 * pg), MB - 128 * (2 * pg + 1)]
                        qkps = qk_ps.tile([128, 2 * MB], F32, tag="qk")
                        off = 0
                        for u in range(2):
                            p = 2 * pg + u
                            j = 4 * i + p
                            w = widths[u]
                            nc.tensor.matmul(
                                qkps[:, off : off + w],
                                lhsT=krow[:, 128 * j : 128 * (j + 1)],
                                rhs=qrow[:, MB * i + 128 * p : MB * (i + 1)],
                                start=True,
                                stop=False,
                                skip_group_check=True,
                            )
                            nc.tensor.matmul(
                                qkps[:, off : off + 128],
                                lhsT=ident,
                                rhs=tri_mask,
                                start=False,
                                stop=True,
                                skip_group_check=True,
                            )
                            off += w
                        pt = pt_pool.tile([128, 2 * MB], BF16, tag="pt")
                        nc.scalar.activation(
                            pt[:, 0:off], qkps[:, 0:off],
                            mybir.ActivationFunctionType.Exp,
                        )
                        off = 0
                        for u in range(2):
                            p = 2 * pg + u
                            j = 4 * i + p
                            w = widths[u]
                            nc.tensor.matmul(
                                pvps[:, 128 * p : MB],
                                lhsT=vj_ap(j, hh),
                                rhs=pt[:, off : off + w],
                                start=(i == 0 and p == 0),
                                stop=(p == 3),
                                skip_group_check=True,
                            )
                            off += w
                    # normalize: o = pv[0:64] / pv[64]
                    # (reciprocal_approx_fast is a custom DVE op; stage the
                    # denominator through SBUF before it)
                    drow = small.tile([1, MB], F32, tag="drow")
                    nc.vector.tensor_copy(drow, pvps[D : D + 1, :])
                    rrow = small.tile([1, MB], F32, tag="rrow")
                    nc.vector.reciprocal_approx_fast(rrow, drow)
                    rbc = small.tile([D, MB], F32, tag="rbc")
                    nc.gpsimd.partition_broadcast(rbc, rrow)
                    if hh < 2:
                        odst = oT_a[D * hh : D * (hh + 1), MB * i : MB * (i + 1)]
                    else:
                        odst = oT_b[:, MB * i : MB * (i + 1)]
                    nc.vector.tensor_mul(odst, pvps[0:D, :], rbc)

            outproj(NMB - 1)
            if debug:
                nc.sync.dma_start(v_dbg, v_sb)
                nc.sync.dma_start(q_dbg, qT[0])
                nc.sync.dma_start(k_dbg, kT[0])
                nc.sync.dma_start(oa_dbg, oT_a)
    nc.compile()
    return nc


GROUPS_HEADS = [[3 * g + k for k in range(NH_CORE)] for g in range(GROUPS)]


def _prep_core_inputs(x, W1, b1, W2):
    """Per-core input dicts. Core index = 4*b + g."""
    mi = _build_masks()
    scale = np.float32(1.0 / np.sqrt(D))  # 1/8, exact in fp32
    bf = ml_dtypes.bfloat16
    in_maps = []
    per_g = []
    for g in range(GROUPS):
        heads = GROUPS_HEADS[g]
        w1qk = np.empty((C, 2 * CC), np.float32)
        w1v = np.empty((C, CC), np.float32)
        bqk = np.zeros((128, NH_CORE), np.float32)
        w2T = np.empty((CC, C), np.float32)
        for hh, h in enumerate(heads):
            w1qk[:, 128 * hh : 128 * hh + D] = (W1[D * h : D * h + D] * scale).T
            w1qk[:, 128 * hh + D : 128 * (hh + 1)] = W1[C + D * h : C + D * h + D].T
            w1v[:, D * hh : D * (hh + 1)] = W1[2 * C + D * h : 2 * C + D * h + D].T
            bqk[0:D, hh] = b1[D * h : D * h + D] * scale
            bqk[D:128, hh] = b1[C + D * h : C + D * h + D]
            w2T[D * hh : D * hh + D] = W2[:, D * h : D * h + D].T
        per_g.append(
            {
                "w1qk": w1qk.astype(bf),
                "w1v": w1v.astype(bf),
                "w2a": np.ascontiguousarray(w2T[0:128]).astype(bf),
                "w2b": np.ascontiguousarray(w2T[128:CC]).astype(bf),
                "bqk": bqk,
                "mi": mi,
            }
        )
    for b in range(B):
        xT = np.ascontiguousarray(np.asarray(x[b]).T).astype(bf)  # (768, 2048)
        for g in range(GROUPS):
            in_maps.append({"x": xT, **per_g[g]})
    return in_maps


_PROGRAM_CACHE = {}


def kernel(x, W1, b1, W2, b2):
    global LAST_RESULTS
    x = np.asarray(x, np.float32)
    W1 = np.asarray(W1, np.float32)
    b1 = np.asarray(b1, np.float32)
    W2 = np.asarray(W2, np.float32)
    b2 = np.asarray(b2, np.float32)

    if "prog" not in _PROGRAM_CACHE:
        _PROGRAM_CACHE["prog"] = _build_program()
    nc = _PROGRAM_CACHE["prog"]

    in_maps = _prep_core_inputs(x, W1, b1, W2)
    trace = os.environ.get("KERNEL_TRACE", "0") == "1"
    res = bass_utils.run_bass_kernel_spmd(
        nc, in_maps, core_ids=list(range(NCORES)), trace=trace
    )
    LAST_RESULTS = res

    # v-bias is linear through attention (softmax weights sum to 1):
    # fold it into the output bias.
    b2p = b2 + W2 @ b1[2 * C : 3 * C]
    out = np.empty((B, T, C), np.float32)
    for b in range(B):
        acc = res.results[GROUPS * b]["outT"].astype(np.float32)
        for g in range(1, GROUPS):
            acc = acc + res.results[GROUPS * b + g]["outT"].astype(np.float32)
        out[b] = acc.T + b2p[None, :]
    return out
